# revision 1
# baseline (speedup 1.0000x reference)
"""Trainium2 Bass kernel for cubic (Keys) interpolation of vertices in a 3D volume.

Strategy (sharding_hint): shard the vertex dim across the 8 NeuronCores;
replicate the (read-only) volume.  The volume is replicated in a
gather-friendly layout: a (dx,dy)-shingled, channel-innermost copy
  S3[x, y, z, dx, dy, c] = vol[c, x+dx, y+dy, z]
so that rows (x,y,z)..(x,y,z+3) — 4 sequential 512B rows = one 2KB run —
hold a vertex's entire 4x4x4x8 neighborhood.  The TRN2 indirect-DMA
(DynamicDGE) gathers one arbitrary-start sequential run per partition per
call, so one call fetches 128 vertices' neighborhoods with exactly 2KB of
HBM traffic per vertex (the memory-roofline minimum).

Per core (18750 vertices, padded to 19200 = 128 partitions x 150 slots):
  prologue: load verts, clip, floor (magic-number), cubic weights w0..w3
            per dim, linear gather indices (all exact in fp32, cast i32).
  per tile of 25 slots/partition (6 tiles):
    - 25 indirect DMAs (one per slot) gather [128, 512] f32 each
    - DVE: multiply by combined weights W[s,k,i,j] (broadcast over c),
      tree-reduce k (z), i (x), j (y), compact, DMA out.
"""

import numpy as np

import concourse.bass as bass
import concourse.tile as tile
from concourse import bacc, mybir
from concourse.bass import IndirectOffsetOnAxis
from concourse.bass_utils import run_bass_kernel_spmd

X, Y, Z, C = 112, 224, 160, 8
P = 128
NCORES = 8
V = 150000
VCORE = V // NCORES          # 18750
NV = 150                     # vertex slots per partition (128*150=19200 >= 18750)
NT = 10                      # slots per partition per tile
NTILES = NV // NT            # 15
GBUFS = 5                    # gather-tile pipelining depth
NROW = X * Y * Z             # shingle rows (128 f32 = 512B each)
MAGIC = 12582912.0           # 1.5 * 2**23: round-to-int magic for fp32
OFFC = float(Y * Z + Z + 1)  # 36001: folds the (-1,-1,-1) corner shift

F32 = mybir.dt.float32
I32 = mybir.dt.int32
ALU = mybir.AluOpType

_CACHE = {}


def _build_program():
    nc = bacc.Bacc("TRN2", target_bir_lowering=False, debug=False,
                   num_devices=NCORES)
    s_in = nc.dram_tensor("shingle", [NROW, 128], F32, kind="ExternalInput").ap()
    vert_in = nc.dram_tensor("vert", [P, NV * 3], F32, kind="ExternalInput").ap()
    out_ext = nc.dram_tensor("out", [P, NV * C], F32, kind="ExternalOutput").ap()

    with tile.TileContext(nc) as tc:
        _emit(tc, out_ext, vert_in, s_in)
    nc.compile()
    return nc


def _emit(tc, out_ext, vert_in, s_in):
    nc = tc.nc
    vec = nc.vector

    with (
        tc.tile_pool(name="keep", bufs=1) as keep,
        tc.tile_pool(name="pro", bufs=1) as pro,
        tc.tile_pool(name="gpool", bufs=GBUFS) as gpool,
        tc.tile_pool(name="wpool", bufs=GBUFS) as wpool,
        tc.tile_pool(name="opool", bufs=GBUFS) as opool,
    ):
        # long-lived across the whole kernel
        wr = keep.tile([P, 4 * NV * 3], F32)    # [i, s, d]
        i1 = keep.tile([P, NV], I32)
        # prologue-only scratch
        vt = pro.tile([P, NV * 3], F32)
        fl = pro.tile([P, NV * 3], F32)
        u = pro.tile([P, NV * 3], F32)
        u2 = pro.tile([P, NV * 3], F32)
        u3 = pro.tile([P, NV * 3], F32)
        tmp = pro.tile([P, NV * 3], F32)
        m1 = pro.tile([P, NV], F32)
        idxf = pro.tile([P, NV], F32)

        nc.sync.dma_start(out=vt[:], in_=vert_in)

        # clip per dim (max_b differs per dim)
        vt3 = vt[:].rearrange("p (s d) -> p s d", d=3)
        for d, dim in enumerate((X, Y, Z)):
            sl = vt3[:, :, d]
            vec.tensor_scalar(out=sl, in0=sl,
                              scalar1=float(np.float32(1.0 + 1e-5)),
                              scalar2=float(np.float32(dim - 2 - 1e-5)),
                              op0=ALU.max, op1=ALU.min)

        # fl = round(v - 0.5) via magic number (== floor except exact-int v,
        # where u becomes 1.0 and the window shifts by one -- same result)
        vec.tensor_scalar(out=fl[:], in0=vt[:], scalar1=0.5, scalar2=MAGIC,
                          op0=ALU.subtract, op1=ALU.add)
        vec.tensor_scalar(out=fl[:], in0=fl[:], scalar1=MAGIC, scalar2=None,
                          op0=ALU.subtract)

        # gather indices FIRST so the Pool engine can start fetching while
        # the weight polynomials are still being computed.
        # linear shingle-row index of the (-1,-1,-1) corner (exact ints < 2^22)
        fl3 = fl[:].rearrange("p (s d) -> p s d", d=3)
        vec.scalar_tensor_tensor(out=m1[:], in0=fl3[:, :, 0], scalar=float(Y),
                                 in1=fl3[:, :, 1], op0=ALU.mult, op1=ALU.add)
        vec.scalar_tensor_tensor(out=idxf[:], in0=m1[:], scalar=float(Z),
                                 in1=fl3[:, :, 2], op0=ALU.mult, op1=ALU.add)
        vec.tensor_scalar(out=idxf[:], in0=idxf[:], scalar1=OFFC, scalar2=None,
                          op0=ALU.subtract)
        vec.tensor_copy(out=i1[:], in_=idxf[:])  # exact-int f32 -> i32

        vec.tensor_tensor(out=u[:], in0=vt[:], in1=fl[:], op=ALU.subtract)
        vec.tensor_tensor(out=u2[:], in0=u[:], in1=u[:], op=ALU.mult)
        vec.tensor_tensor(out=u3[:], in0=u2[:], in1=u[:], op=ALU.mult)

        # raw weights (2x the Keys weights; the 3 raw factors carry 8x,
        # compensated by folding 0.125 into the y weights below)
        wr4 = wr[:].rearrange("p (i e) -> p i e", i=4)
        w0, w1, w2, w3 = (wr4[:, i] for i in range(4))
        # w0r = -u3 + 2u2 - u
        vec.tensor_tensor(out=tmp[:], in0=u3[:], in1=u[:], op=ALU.add)
        vec.scalar_tensor_tensor(out=w0, in0=u2[:], scalar=2.0, in1=tmp[:],
                                 op0=ALU.mult, op1=ALU.subtract)
        # w1r = 3u3 - (5u2 - 2)
        vec.tensor_scalar(out=tmp[:], in0=u2[:], scalar1=5.0, scalar2=2.0,
                          op0=ALU.mult, op1=ALU.subtract)
        vec.scalar_tensor_tensor(out=w1, in0=u3[:], scalar=3.0, in1=tmp[:],
                                 op0=ALU.mult, op1=ALU.subtract)
        # w2r = -3u3 + (4u2 + u)
        vec.scalar_tensor_tensor(out=tmp[:], in0=u2[:], scalar=4.0, in1=u[:],
                                 op0=ALU.mult, op1=ALU.add)
        vec.scalar_tensor_tensor(out=w2, in0=u3[:], scalar=-3.0, in1=tmp[:],
                                 op0=ALU.mult, op1=ALU.add)
        # w3r = u3 - u2
        vec.tensor_tensor(out=w3, in0=u3[:], in1=u2[:], op=ALU.subtract)
        # fold 1/8 into all y weights
        wr_isd = wr[:].rearrange("p (i s d) -> p i s d", i=4, s=NV, d=3)
        wy_all = wr_isd[:, :, :, 1]
        vec.tensor_scalar(out=wy_all, in0=wy_all, scalar1=0.125, scalar2=None,
                          op0=ALU.mult)

        for t in range(NTILES):
            s0 = t * NT
            G = gpool.tile([P, NT * 512], F32, tag="G")
            Wt = wpool.tile([P, NT * 64], F32, tag="W")
            w02 = wpool.tile([P, NT * 16], F32, tag="w02")
            ot = opool.tile([P, NT * C], F32, tag="ot")

            # one indirect DMA per slot: 128 partitions x one 2KB run each
            for s in range(NT):
                nc.gpsimd.indirect_dma_start(
                    out=G[:, s * 512:(s + 1) * 512],
                    out_offset=None,
                    in_=s_in,
                    in_offset=IndirectOffsetOnAxis(
                        ap=i1[:, s0 + s:s0 + s + 1], axis=0),
                )

            wx = wr_isd[:, :, s0:s0 + NT, 0]   # [p, i, s]
            wz = wr_isd[:, :, s0:s0 + NT, 2]
            wy = wr_isd[:, :, s0:s0 + NT, 1]

            # payload layout per slot: [k(z):4, i(dx):4, j(dy):4, c:8]
            # w02[s, k, i] = wz[k, s] * wx[i, s]
            w02v = w02[:].rearrange("p (s k i) -> p s k i", s=NT, k=4, i=4)
            vec.tensor_tensor(
                out=w02v,
                in0=wz.transpose([0, 2, 1]).unsqueeze(3).to_broadcast([P, NT, 4, 4]),
                in1=wx.transpose([0, 2, 1]).unsqueeze(2).to_broadcast([P, NT, 4, 4]),
                op=ALU.mult)
            # W[s, ki, j] = w02[s, ki] * wy[j, s]
            Wv = Wt[:].rearrange("p (s e j) -> p s e j", s=NT, e=16, j=4)
            vec.tensor_tensor(
                out=Wv,
                in0=w02[:].rearrange("p (s e) -> p s e", s=NT)
                    .unsqueeze(3).to_broadcast([P, NT, 16, 4]),
                in1=wy.transpose([0, 2, 1]).unsqueeze(2).to_broadcast([P, NT, 16, 4]),
                op=ALU.mult)

            # G *= W  (broadcast over c)
            Gv = G[:].rearrange("p (s e c) -> p s e c", s=NT, e=64, c=8)
            vec.tensor_tensor(
                out=Gv, in0=Gv,
                in1=Wt[:].rearrange("p (s e) -> p s e", s=NT)
                    .unsqueeze(3).to_broadcast([P, NT, 64, 8]),
                op=ALU.mult)

            # tree-reduce k (stride 128), i (stride 32), j (stride 8)
            Gk = G[:].rearrange("p (s k r) -> p s k r", s=NT, k=4, r=128)
            vec.tensor_tensor(out=Gk[:, :, 0], in0=Gk[:, :, 0], in1=Gk[:, :, 1], op=ALU.add)
            vec.tensor_tensor(out=Gk[:, :, 2], in0=Gk[:, :, 2], in1=Gk[:, :, 3], op=ALU.add)
            vec.tensor_tensor(out=Gk[:, :, 0], in0=Gk[:, :, 0], in1=Gk[:, :, 2], op=ALU.add)
            Gi = G[:].rearrange("p (s k i r) -> p s k i r", s=NT, k=4, i=4, r=32)[:, :, 0]
            vec.tensor_tensor(out=Gi[:, :, 0], in0=Gi[:, :, 0], in1=Gi[:, :, 1], op=ALU.add)
            vec.tensor_tensor(out=Gi[:, :, 2], in0=Gi[:, :, 2], in1=Gi[:, :, 3], op=ALU.add)
            vec.tensor_tensor(out=Gi[:, :, 0], in0=Gi[:, :, 0], in1=Gi[:, :, 2], op=ALU.add)
            Gj = G[:].rearrange("p (s k i j c) -> p s k i j c",
                                s=NT, k=4, i=4, j=4, c=8)[:, :, 0, 0]
            vec.tensor_tensor(out=Gj[:, :, 0], in0=Gj[:, :, 0], in1=Gj[:, :, 1], op=ALU.add)
            vec.tensor_tensor(out=Gj[:, :, 2], in0=Gj[:, :, 2], in1=Gj[:, :, 3], op=ALU.add)
            vec.tensor_tensor(out=Gj[:, :, 0], in0=Gj[:, :, 0], in1=Gj[:, :, 2], op=ALU.add)

            vec.tensor_copy(out=ot[:], in_=Gj[:, :, 0])
            nc.sync.dma_start(out=out_ext[:, s0 * C:(s0 + NT) * C], in_=ot[:])


def _get_program():
    if "nc" not in _CACHE:
        _CACHE["nc"] = _build_program()
    return _CACHE["nc"]


def _build_shingle(vol):
    """S3[x, y, z, dx, dy, c] = vol[c, x+dx, y+dy, z], flattened [NROW, 128]."""
    volT = np.ascontiguousarray(vol[0].transpose(1, 2, 3, 0))  # (X,Y,Z,C)
    S = np.zeros((X, Y, Z, 4, 4, C), np.float32)
    for dx in range(4):
        for dy in range(4):
            S[:X - dx, :Y - dy, :, dx, dy, :] = volT[dx:, dy:, :, :]
    return S.reshape(NROW, 128)


def _make_in_maps(vert, vol, n_cores=NCORES):
    Sf = _build_shingle(np.asarray(vol, dtype=np.float32))
    vert = np.asarray(vert, dtype=np.float32)
    in_maps = []
    for c in range(n_cores):
        vpad = np.zeros((P * NV, 3), np.float32)
        vpad[:VCORE] = vert[0, c * VCORE:(c + 1) * VCORE]
        in_maps.append({"vert": np.ascontiguousarray(vpad.reshape(P, NV * 3)),
                        "shingle": Sf})
    return in_maps


def run_cores(vert, vol, trace=False, n_cores=NCORES, **kwargs):
    nc = _get_program()
    res = run_bass_kernel_spmd(nc, _make_in_maps(vert, vol, n_cores),
                               list(range(n_cores)), trace=trace, **kwargs)
    outs = [np.asarray(r["out"]).reshape(P * NV, C)[:VCORE]
            for r in res.results]
    full = np.concatenate(outs, axis=0)[None]  # (1, n_cores*VCORE, C)
    return full, res


def kernel(vert, vol):
    full, _ = run_cores(vert, vol, trace=False)
    return full

